# revision 9
# baseline (speedup 1.0000x reference)
"""Chamfer distance kernel for Trainium2 (8 NeuronCores, SPMD).

Reference computation:
    p1 = pc1.reshape(-1, 3)  [N1=16384, 3]
    p2 = pc2.reshape(-1, 3)  [N2=16384, 3]
    d[i, j] = ||p1_i - p2_j||
    out = mean_j(min_i d[i,j]) + mean_i(min_j d[i,j])

Strategy (v2 -- single-orientation, both mins from the same tile):
  - Shard pc2 rows across 8 cores (2048 points each). Each core computes
    its [2048 pc2, 16384 pc1] block of SCALE*d2 ONCE, as 16x8 PSUM tiles
    [128 pc2-part, 2048 pc1-free], and extracts BOTH reductions from it:
      row path (dist1): min over the free dim + over the 8 pc1 groups
        -> complete row-min for this core's 2048 pc2 points.
      col path (dist2): elementwise min accumulate over the 16 pc2
        blocks -> [128, 16384] partial col-min; partition/core mins on
        the host.
    This halves PE work vs computing two orientations, and -- the real
    win -- ONE ScalarE PSUM->SBUF fp16 copy per tile feeds both paths.
  - SCALE*d2 is produced by one K=24 augmented matmul per [128, 1024]
    chunk (compensated bf16 hi/mid/lo splits; error ~2.5e-7).
  - PE 2-wide row-group concurrency: consecutive pc2 blocks (bj even/odd)
    use lhsT at partition bases 0/32, so their matmuls run in different
    32-row groups of the PE array concurrently (~2x matmul throughput).
  - Per tile: ScalarE copies PSUM -> fp16 slice of a per-bj row buffer
    (~1.9us); DVE accumulates the col path with one fp16 tensor_tensor
    min at 2x rate (~1.1us). Per bj, after all 8 groups land: DVE runs a
    fold chain (6 pairwise mins at 2x + one short reduce, ~9us) over the
    [128, 16384] buffer for the row-min.
  - fp16 overflow on large d2 saturates to ~inf which is harmless under
    min; the x512 pre-scale keeps the relevant small d2 in normal range.
  - Col accumulators ship as [128, 16384] fp16 (partition+core mins on
    host, where they're cheap); row mins ship as [128, 16] fp32.
  - Walrus accepts only one sem-wait per compute instruction; Tile emits
    more on recycled tile slots. _legalize_waits strips transitively
    implied same-engine waits and splits the rest onto injected NoOps.
"""

import os
import sys

import numpy as np

for _p in ("/opt/trn_rl_repo",):
    if os.path.isdir(_p) and _p not in sys.path:
        sys.path.append(_p)

import ml_dtypes

import concourse.bass as bass
import concourse.mybir as mybir
import concourse.tile as tile
from concourse.bass_utils import run_bass_kernel_spmd

BF16 = ml_dtypes.bfloat16

N_CORES = 8
N1 = 16384            # total pc1 points
N_SHARD = 2048        # pc2 points per core
N_GROUPS = 8          # pc1 column-groups
GROUP_COLS = N1 // N_GROUPS  # 2048
N_BLOCKS = N_SHARD // 128    # 16 pc2 blocks per core
K = 24                # augmented contraction depth
MM_N = 512            # matmul moving free dim (one PSUM bank of fp32)
SCALE = 512.0         # power-of-two scale on d2 (fp16 normal range)
GP_EVERY = 0          # GpSimd can't run TENSOR_TENSOR on TRN2 (ISA check)
IN_COLS = N1 + N_SHARD  # packed input columns: [p1 moving | p2 weights]

TRACE = False         # test harness can flip this for profiled runs
LAST_RESULTS = None   # stashed BassKernelResults for the test harness

_NC_CACHE = None


def _build_nc():
    """Build the per-core Bass module (same NEFF on all 8 cores)."""
    nc = bass.Bass(trn_type="TRN2")

    # Packed input, cols: [0:16384) p1m (moving side, bases 0+32),
    # [16384:18432) p2w (weight side, bases 0+32).
    inp = nc.dram_tensor("inp", [128, IN_COLS], mybir.dt.bfloat16,
                         kind="ExternalInput")
    # mrow[p, bj] = min over ALL pc1 of SCALE*d2 for pc2 point bj*128+p.
    mrow = nc.dram_tensor("mrow", [128, N_BLOCKS], mybir.dt.float32,
                          kind="ExternalOutput")
    # mcol[p, c] = min over this core's pc2 blocks (partition p within
    # each block) of SCALE*d2 vs pc1 point c; host mins partitions+cores.
    mcol = nc.dram_tensor("mcol", [128, N1], mybir.dt.float16,
                          kind="ExternalOutput")

    MIN = mybir.AluOpType.min

    with tile.TileContext(nc) as tc:
        with (
            tc.tile_pool(name="ins", bufs=1) as ins_pool,
            tc.tile_pool(name="psum", bufs=4, space="PSUM") as psum_pool,
            tc.tile_pool(name="rbuf", bufs=3) as rbuf_pool,
            tc.tile_pool(name="cacc", bufs=1) as cacc_pool,
            tc.tile_pool(name="outs", bufs=1) as out_pool,
        ):
            inp_sb = ins_pool.tile([128, IN_COLS], mybir.dt.bfloat16,
                                   tag="inp")
            # p2w (small, needed by every matmul) on its own queue first;
            # p1m split across 4 more queues so loads run concurrently.
            nc.sync.dma_start(inp_sb[:, N1:IN_COLS], inp[:, N1:IN_COLS])
            q = N1 // 4
            for qi in range(4):
                nc.sync.dma_start(inp_sb[:, qi * q:(qi + 1) * q],
                                  inp[:, qi * q:(qi + 1) * q])
            p1m = inp_sb[:, 0:N1]
            p2w = inp_sb[:, N1:IN_COLS]

            CA = cacc_pool.tile([128, N1], mybir.dt.float16, tag="cacc")
            mrow_sb = out_pool.tile([128, N_BLOCKS], mybir.dt.float32,
                                    tag="mrow")

            # Row-path fold ops for a finished pair are emitted
            # interleaved between the NEXT pair's col-min ops, so the
            # in-order DVE queue always has ready work while ACT copies
            # land (the fold inputs were produced a whole pair earlier).
            pending = []

            def emit_fold_ops(R, bjs):
                for h in range(2):
                    buf, bj = R[h], bjs[h]
                    w = N1 // 2
                    while w >= 256:
                        pending.append((
                            lambda buf=buf, w=w: nc.vector.tensor_tensor(
                                out=buf[:, :w], in0=buf[:, :w],
                                in1=buf[:, w:2 * w], op=MIN,
                            )))
                        w //= 2
                    pending.append((
                        lambda buf=buf, bj=bj: nc.vector.tensor_reduce(
                            out=mrow_sb[:, bj:bj + 1], in_=buf[:, :256],
                            axis=mybir.AxisListType.X, op=MIN,
                        )))

            HALF = GROUP_COLS // 2  # psum tile width (2 banks)

            for pr in range(N_BLOCKS // 2):
                bjs = (2 * pr, 2 * pr + 1)
                R = [rbuf_pool.tile([128, N1], mybir.dt.float16,
                                    tag="rbuf", name=f"R{pr}_{h}")
                     for h in range(2)]
                for g in range(N_GROUPS):
                    # 4 psum tiles [128, 1024] -> PE can run a half-group
                    # ahead of ScalarE's eviction (no ping-pong stall).
                    pts = [psum_pool.tile([128, HALF], mybir.dt.float32,
                                          tag="ps", name=f"ps{pr}_{g}_{i}")
                           for i in range(4)]  # [h0A, h0B, h1A, h1B]
                    # Interleave the two blocks' matmuls: lhsT at bases
                    # 0/32 -> different PE row groups -> they overlap.
                    for half in range(2):
                        for c in range(HALF // MM_N):
                            for h in range(2):
                                b = 32 * h
                                col0 = (g * GROUP_COLS + half * HALF
                                        + c * MM_N)
                                nc.tensor.matmul(
                                    pts[2 * h + half][:, c * MM_N:
                                                      (c + 1) * MM_N],
                                    p2w[b:b + K,
                                        bjs[h] * 128:(bjs[h] + 1) * 128],
                                    p1m[b:b + K, col0:col0 + MM_N],
                                    start=True, stop=True,
                                )
                    for h in range(2):
                        bj = bjs[h]
                        for half in range(2):
                            rsl = R[h][:, g * GROUP_COLS + half * HALF:
                                       g * GROUP_COLS + (half + 1) * HALF]
                            nc.scalar.copy(rsl, pts[2 * h + half][:])
                        rsl = R[h][:, g * GROUP_COLS:(g + 1) * GROUP_COLS]
                        csl = CA[:, g * GROUP_COLS:(g + 1) * GROUP_COLS]
                        if bj == 0:
                            nc.vector.tensor_copy(csl, rsl)
                        else:
                            nc.vector.tensor_tensor(out=csl, in0=csl,
                                                    in1=rsl, op=MIN)
                        if pending:
                            pending.pop(0)()
                # queue this pair's row-path folds for the next pair
                emit_fold_ops(R, bjs)
            while pending:
                pending.pop(0)()

            nc.sync.dma_start(mrow[:, :], mrow_sb[:])
            for qi in range(4):
                nc.sync.dma_start(mcol[:, qi * q:(qi + 1) * q],
                                  CA[:, qi * q:(qi + 1) * q])

    _legalize_waits(nc)
    return nc


def _legalize_waits(nc):
    """Walrus's per-instruction structs carry at most one sem-wait, but
    Tile's sem assignment can emit several (slot-recycle WAR + input RAW).

    1. Same-engine waits are dropped when a cross-engine wait remains:
       engines execute in order and the cross-engine consumer they wait
       on transitively waited on those same-engine ticks.
    2. The kernel-tail Drain waits on every DMA queue + PE + DVE; all of
       it is transitively covered by the output DMAs.
    3. Any instruction still carrying N>1 waits gets N-1 same-engine
       NoOps injected right before it, one overflow wait each.
    """
    import concourse.mybir as mybir

    blocks = nc.m.functions[0].blocks

    # 1. same-engine strip
    for blk in blocks:
        for ins in blk.instructions:
            si = ins.sync_info
            if si is None or len(si.on_wait) <= 1 or not si.on_update:
                continue
            self_eng = si.on_update[0].ant_name.split("_")[0]
            keep = [w for w in si.on_wait
                    if w.ant_name.split("_")[0] != self_eng]
            if keep and len(keep) < len(si.on_wait):
                si.on_wait = keep
                ins.sync_info = si

    # 2. tail drain: keep only the output DMA queues' waits
    out_sems = set()
    for blk in blocks:
        for ins in blk.instructions:
            if type(ins).__name__ == "InstDMACopy" and ins.outs and \
                    getattr(ins.outs[0], "memref", "") in ("mrow", "mcol"):
                si = ins.sync_info
                for u in (si.on_update if si else []):
                    out_sems.add(u.ant_name)
    for blk in blocks:
        for ins in blk.instructions:
            if type(ins).__name__ != "InstDrain" or not out_sems:
                continue
            si = ins.sync_info
            if si is None or len(si.on_wait) <= 1:
                continue
            keep = [w for w in si.on_wait if w.ant_name in out_sems]
            if keep and len(keep) < len(si.on_wait):
                si.on_wait = keep
                ins.sync_info = si

    # 3. split remaining multi-waits onto same-engine NoOps
    eng_by_prefix = {
        "PE": mybir.EngineType.PE,
        "DVE": mybir.EngineType.DVE,
        "ACT": mybir.EngineType.Activation,
        "POOL": mybir.EngineType.Pool,
        "SP": mybir.EngineType.SP,
    }
    nop_id = [0]
    for blk in blocks:
        new_list = []
        changed = False
        for ins in blk.instructions:
            si = ins.sync_info
            if si is not None and len(si.on_wait) > 1:
                eng = getattr(ins, "engine", None)
                if eng is None and si.on_update:
                    eng = eng_by_prefix.get(
                        si.on_update[0].ant_name.split("_")[0])
                assert eng is not None, \
                    f"{ins.name}: cannot infer engine for wait split"
                waits = list(si.on_wait)
                for w in waits[:-1]:
                    nop_id[0] += 1
                    nop = mybir.InstNoOp(
                        name=f"I-waitnop-{nop_id[0]}", ins=[], outs=[],
                        engine=eng,
                        sync_info=mybir.SyncInfo(on_wait=[w], on_update=[]),
                    )
                    new_list.append(nop)
                si.on_wait = [waits[-1]]
                ins.sync_info = si
                changed = True
            new_list.append(ins)
        if changed:
            blk.instructions = new_list


def _split3(x):
    """fp32 -> three bf16 terms with x ~= h + m + l (residual ~2^-24 |x|)."""
    h = x.astype(BF16)
    r = x - h.astype(np.float32)
    m = r.astype(BF16)
    l = (r - m.astype(np.float32)).astype(BF16)
    return h, m, l


def _prep_side(p):
    """p: [N, 3] fp32 -> (weight_rows [24, N], moving_rows [24, N]).

    Row r of the weight side pairs with row r of the other cloud's moving
    side; the contraction sums, per coordinate, the six hi/mid/lo product
    terms of magnitude >= ~2^-17 (double-compensated bf16 dot, error
    ~2.5e-7), plus three hi/mid/lo rows for each side's |p|^2. The weight
    side carries SCALE (a power of two), so PSUM holds SCALE*d2 exactly
    scaled -- keeping d2 row-mins (~2.5e-5 here) inside fp16 normal range
    for the fp16 min data path.
    """
    x, y, z = p[:, 0], p[:, 1], p[:, 2]
    sq = (x * x + y * y + z * z).astype(np.float32)
    w_rows, m_rows = [], []
    for c in (x, y, z):
        h, m, l = _split3(c)
        # (W, M) pairs: (h,h) (m,h) (h,m) (l,h) (m,m) (h,l)
        w_rows += [-2 * SCALE * h, -2 * SCALE * m, -2 * SCALE * h,
                   -2 * SCALE * l, -2 * SCALE * m, -2 * SCALE * h]
        m_rows += [h, h, m, h, m, l]
    ones = np.ones_like(sq)
    w_rows += [SCALE * ones] * 3 + list(_split3(SCALE * sq))
    m_rows += list(_split3(sq)) + [ones] * 3
    return (np.stack(w_rows).astype(BF16), np.stack(m_rows).astype(BF16))


def _dual_base(rows):
    """[24, N] -> [128, N] with copies at partition bases 0 and 32."""
    out = np.zeros((128, rows.shape[1]), dtype=BF16)
    out[0:K] = rows
    out[32:32 + K] = rows
    return out


def kernel(pc1, pc2):
    global _NC_CACHE, LAST_RESULTS
    p1 = np.asarray(pc1, dtype=np.float32).reshape(-1, 3)
    p2 = np.asarray(pc2, dtype=np.float32).reshape(-1, 3)
    assert p1.shape == (N1, 3) and p2.shape == (N_CORES * N_SHARD, 3)

    p1m_np = _dual_base(_prep_side(p1)[1])

    in_maps = []
    for c in range(N_CORES):
        shard = p2[c * N_SHARD:(c + 1) * N_SHARD]
        p2w_np = _dual_base(_prep_side(shard)[0])
        packed = np.concatenate([p1m_np, p2w_np], axis=1)
        in_maps.append({"inp": np.ascontiguousarray(packed)})

    if _NC_CACHE is None:
        _NC_CACHE = _build_nc()

    res = run_bass_kernel_spmd(
        _NC_CACHE, in_maps, core_ids=list(range(N_CORES)), trace=TRACE,
    )
    LAST_RESULTS = res

    # mrow per core: complete row-mins of SCALE*d2 for its 2048 pc2 pts.
    # mcol per core: partial col-mins -> min partitions, then cores.
    d2_1 = np.concatenate(
        [r["mrow"].T.reshape(-1) for r in res.results])          # [16384]
    d2_2 = np.min(
        np.stack([r["mcol"].astype(np.float32).min(axis=0)
                  for r in res.results]), axis=0)                # [16384]

    dist1 = np.sqrt(np.maximum(d2_1 / SCALE, 0.0))
    dist2 = np.sqrt(np.maximum(d2_2 / SCALE, 0.0))
    return np.asarray(dist1.mean() + dist2.mean(), dtype=np.float32)


# revision 11
# speedup vs baseline: 1.2358x; 1.2358x over previous
"""Chamfer distance kernel for Trainium2 (8 NeuronCores, SPMD).

Reference computation:
    p1 = pc1.reshape(-1, 3)  [N1=16384, 3]
    p2 = pc2.reshape(-1, 3)  [N2=16384, 3]
    d[i, j] = ||p1_i - p2_j||
    out = mean_j(min_i d[i,j]) + mean_i(min_j d[i,j])

Strategy (v2 -- single-orientation, both mins from the same tile):
  - Shard pc2 rows across 8 cores (2048 points each). Each core computes
    its [2048 pc2, 16384 pc1] block of SCALE*d2 ONCE, as 16x8 PSUM tiles
    [128 pc2-part, 2048 pc1-free], and extracts BOTH reductions from it:
      row path (dist1): min over the free dim + over the 8 pc1 groups
        -> complete row-min for this core's 2048 pc2 points.
      col path (dist2): elementwise min accumulate over the 16 pc2
        blocks -> [128, 16384] partial col-min; partition/core mins on
        the host.
    This halves PE work vs computing two orientations, and -- the real
    win -- ONE ScalarE PSUM->SBUF fp16 copy per tile feeds both paths.
  - SCALE*d2 is produced by one K=24 augmented matmul per [128, 1024]
    chunk (compensated bf16 hi/mid/lo splits; error ~2.5e-7).
  - PE 2-wide row-group concurrency: consecutive pc2 blocks (bj even/odd)
    use lhsT at partition bases 0/32, so their matmuls run in different
    32-row groups of the PE array concurrently (~2x matmul throughput).
  - Per tile: ScalarE copies PSUM -> fp16 slice of a per-bj row buffer
    (~1.9us); DVE accumulates the col path with one fp16 tensor_tensor
    min at 2x rate (~1.1us). Per bj, after all 8 groups land: DVE runs a
    fold chain (6 pairwise mins at 2x + one short reduce, ~9us) over the
    [128, 16384] buffer for the row-min.
  - fp16 overflow on large d2 saturates to ~inf which is harmless under
    min; the x512 pre-scale keeps the relevant small d2 in normal range.
  - Col accumulators ship as [128, 16384] fp16 (partition+core mins on
    host, where they're cheap); row mins ship as [128, 16] fp32.
  - Walrus accepts only one sem-wait per compute instruction; Tile emits
    more on recycled tile slots. _legalize_waits strips transitively
    implied same-engine waits and splits the rest onto injected NoOps.
"""

import os
import sys

import numpy as np

for _p in ("/opt/trn_rl_repo",):
    if os.path.isdir(_p) and _p not in sys.path:
        sys.path.append(_p)

import ml_dtypes

import concourse.bass as bass
import concourse.mybir as mybir
import concourse.tile as tile
from concourse.bass_utils import run_bass_kernel_spmd

BF16 = ml_dtypes.bfloat16

N_CORES = 8
N1 = 16384            # total pc1 points
N_SHARD = 2048        # pc2 points per core
N_GROUPS = 8          # pc1 column-groups
GROUP_COLS = N1 // N_GROUPS  # 2048
N_BLOCKS = N_SHARD // 128    # 16 pc2 blocks per core
K = 24                # augmented contraction depth
MM_N = 512            # matmul moving free dim (one PSUM bank of fp32)
SCALE = 512.0         # power-of-two scale on d2 (fp16 normal range)
GP_EVERY = 0          # GpSimd can't run TENSOR_TENSOR on TRN2 (ISA check)
IN_COLS = N1 + N_SHARD  # packed input columns: [p1 moving | p2 weights]

TRACE = False         # test harness can flip this for profiled runs
LAST_RESULTS = None   # stashed BassKernelResults for the test harness

_NC_CACHE = None


def _build_nc():
    """Build the per-core Bass module (same NEFF on all 8 cores)."""
    nc = bass.Bass(trn_type="TRN2")

    # Packed input, cols: [0:16384) p1m (moving side, bases 0+32),
    # [16384:18432) p2w (weight side, bases 0+32).
    inp = nc.dram_tensor("inp", [128, IN_COLS], mybir.dt.bfloat16,
                         kind="ExternalInput")
    # mrow[p, bj] = min over ALL pc1 of SCALE*d2 for pc2 point bj*128+p.
    mrow = nc.dram_tensor("mrow", [128, N_BLOCKS], mybir.dt.float32,
                          kind="ExternalOutput")
    # mcol[p, c] = min over this core's pc2 blocks (partition p within
    # each block) of SCALE*d2 vs pc1 point c; host mins partitions+cores.
    mcol = nc.dram_tensor("mcol", [128, N1], mybir.dt.float16,
                          kind="ExternalOutput")

    MIN = mybir.AluOpType.min

    with tile.TileContext(nc) as tc:
        with (
            tc.tile_pool(name="ins", bufs=1) as ins_pool,
            tc.tile_pool(name="psum", bufs=2, space="PSUM") as psum_pool,
            tc.tile_pool(name="rbuf", bufs=4) as rbuf_pool,
            tc.tile_pool(name="cacc", bufs=1) as cacc_pool,
            tc.tile_pool(name="outs", bufs=1) as out_pool,
        ):
            inp_sb = ins_pool.tile([128, IN_COLS], mybir.dt.bfloat16,
                                   tag="inp")
            # p2w (small, needed by every matmul) on its own queue first;
            # p1m split across 4 more queues so loads run concurrently.
            nc.sync.dma_start(inp_sb[:, N1:IN_COLS], inp[:, N1:IN_COLS])
            q = N1 // 4
            for qi in range(4):
                nc.sync.dma_start(inp_sb[:, qi * q:(qi + 1) * q],
                                  inp[:, qi * q:(qi + 1) * q])
            p1m = inp_sb[:, 0:N1]
            p2w = inp_sb[:, N1:IN_COLS]

            CA = cacc_pool.tile([128, N1], mybir.dt.float16, tag="cacc")
            mrow_sb = out_pool.tile([128, N_BLOCKS], mybir.dt.float32,
                                    tag="mrow")

            def emit_fold_ops(R, bjs):
                """Row path: fold a finished pair's buffers in place
                (emitted at the NEXT pair's start so the in-order DVE
                queue has ready work while this pair's copies land).
                Safe vs the col path: the col TT reads R before these
                writes (WAR tracked)."""
                for h in range(2):
                    buf, bj = R[h], bjs[h]
                    w = N1 // 2
                    while w >= 512:
                        nc.vector.tensor_tensor(
                            out=buf[:, :w], in0=buf[:, :w],
                            in1=buf[:, w:2 * w], op=MIN,
                        )
                        w //= 2
                    nc.vector.tensor_reduce(
                        out=mrow_sb[:, bj:bj + 1], in_=buf[:, :512],
                        axis=mybir.AxisListType.X, op=MIN,
                    )

            prev = None
            for pr in range(N_BLOCKS // 2):
                bjs = (2 * pr, 2 * pr + 1)
                R = [rbuf_pool.tile([128, N1], mybir.dt.float16,
                                    tag="rbuf", name=f"R{pr}_{h}")
                     for h in range(2)]
                if prev is not None:
                    emit_fold_ops(*prev)
                for g in range(N_GROUPS):
                    pts = [psum_pool.tile([128, GROUP_COLS],
                                          mybir.dt.float32, tag="ps",
                                          name=f"ps{pr}_{g}_{h}")
                           for h in range(2)]
                    # Interleave the two blocks' matmuls: lhsT at bases
                    # 0/32 -> different PE row groups -> drain overlap.
                    for c in range(GROUP_COLS // MM_N):
                        for h in range(2):
                            b = 32 * h
                            nc.tensor.matmul(
                                pts[h][:, c * MM_N:(c + 1) * MM_N],
                                p2w[b:b + K,
                                    bjs[h] * 128:(bjs[h] + 1) * 128],
                                p1m[b:b + K,
                                    g * GROUP_COLS + c * MM_N:
                                    g * GROUP_COLS + (c + 1) * MM_N],
                                start=True, stop=True,
                            )
                    for h in range(2):
                        nc.scalar.copy(
                            R[h][:, g * GROUP_COLS:(g + 1) * GROUP_COLS],
                            pts[h][:])
                # Col path: one full-width fp16 min per block (big DVE
                # ops amortize the ~200ns per-op overhead).
                for h in range(2):
                    if bjs[h] == 0:
                        nc.vector.tensor_copy(CA[:, :], R[h][:, :])
                    else:
                        nc.vector.tensor_tensor(out=CA[:, :], in0=CA[:, :],
                                                in1=R[h][:, :], op=MIN)
                prev = (R, bjs)
            emit_fold_ops(*prev)

            nc.sync.dma_start(mrow[:, :], mrow_sb[:])
            for qi in range(4):
                nc.sync.dma_start(mcol[:, qi * q:(qi + 1) * q],
                                  CA[:, qi * q:(qi + 1) * q])

    _legalize_waits(nc)
    return nc


def _legalize_waits(nc):
    """Walrus's per-instruction structs carry at most one sem-wait, but
    Tile's sem assignment can emit several (slot-recycle WAR + input RAW).

    1. Same-engine waits are dropped when a cross-engine wait remains:
       engines execute in order and the cross-engine consumer they wait
       on transitively waited on those same-engine ticks.
    2. The kernel-tail Drain waits on every DMA queue + PE + DVE; all of
       it is transitively covered by the output DMAs.
    3. Any instruction still carrying N>1 waits gets N-1 same-engine
       NoOps injected right before it, one overflow wait each.
    """
    import concourse.mybir as mybir

    blocks = nc.m.functions[0].blocks

    # 1. same-engine strip
    for blk in blocks:
        for ins in blk.instructions:
            si = ins.sync_info
            if si is None or len(si.on_wait) <= 1 or not si.on_update:
                continue
            self_eng = si.on_update[0].ant_name.split("_")[0]
            keep = [w for w in si.on_wait
                    if w.ant_name.split("_")[0] != self_eng]
            if keep and len(keep) < len(si.on_wait):
                si.on_wait = keep
                ins.sync_info = si

    # 2. tail drain: keep only the output DMA queues' waits
    out_sems = set()
    for blk in blocks:
        for ins in blk.instructions:
            if type(ins).__name__ == "InstDMACopy" and ins.outs and \
                    getattr(ins.outs[0], "memref", "") in ("mrow", "mcol"):
                si = ins.sync_info
                for u in (si.on_update if si else []):
                    out_sems.add(u.ant_name)
    for blk in blocks:
        for ins in blk.instructions:
            if type(ins).__name__ != "InstDrain" or not out_sems:
                continue
            si = ins.sync_info
            if si is None or len(si.on_wait) <= 1:
                continue
            keep = [w for w in si.on_wait if w.ant_name in out_sems]
            if keep and len(keep) < len(si.on_wait):
                si.on_wait = keep
                ins.sync_info = si

    # 3. split remaining multi-waits onto same-engine NoOps
    eng_by_prefix = {
        "PE": mybir.EngineType.PE,
        "DVE": mybir.EngineType.DVE,
        "ACT": mybir.EngineType.Activation,
        "POOL": mybir.EngineType.Pool,
        "SP": mybir.EngineType.SP,
    }
    nop_id = [0]
    for blk in blocks:
        new_list = []
        changed = False
        for ins in blk.instructions:
            si = ins.sync_info
            if si is not None and len(si.on_wait) > 1:
                eng = getattr(ins, "engine", None)
                if eng is None and si.on_update:
                    eng = eng_by_prefix.get(
                        si.on_update[0].ant_name.split("_")[0])
                assert eng is not None, \
                    f"{ins.name}: cannot infer engine for wait split"
                waits = list(si.on_wait)
                for w in waits[:-1]:
                    nop_id[0] += 1
                    nop = mybir.InstNoOp(
                        name=f"I-waitnop-{nop_id[0]}", ins=[], outs=[],
                        engine=eng,
                        sync_info=mybir.SyncInfo(on_wait=[w], on_update=[]),
                    )
                    new_list.append(nop)
                si.on_wait = [waits[-1]]
                ins.sync_info = si
                changed = True
            new_list.append(ins)
        if changed:
            blk.instructions = new_list


def _split3(x):
    """fp32 -> three bf16 terms with x ~= h + m + l (residual ~2^-24 |x|)."""
    h = x.astype(BF16)
    r = x - h.astype(np.float32)
    m = r.astype(BF16)
    l = (r - m.astype(np.float32)).astype(BF16)
    return h, m, l


def _prep_side(p):
    """p: [N, 3] fp32 -> (weight_rows [24, N], moving_rows [24, N]).

    Row r of the weight side pairs with row r of the other cloud's moving
    side; the contraction sums, per coordinate, the six hi/mid/lo product
    terms of magnitude >= ~2^-17 (double-compensated bf16 dot, error
    ~2.5e-7), plus three hi/mid/lo rows for each side's |p|^2. The weight
    side carries SCALE (a power of two), so PSUM holds SCALE*d2 exactly
    scaled -- keeping d2 row-mins (~2.5e-5 here) inside fp16 normal range
    for the fp16 min data path.
    """
    x, y, z = p[:, 0], p[:, 1], p[:, 2]
    sq = (x * x + y * y + z * z).astype(np.float32)
    w_rows, m_rows = [], []
    for c in (x, y, z):
        h, m, l = _split3(c)
        # (W, M) pairs: (h,h) (m,h) (h,m) (l,h) (m,m) (h,l)
        w_rows += [-2 * SCALE * h, -2 * SCALE * m, -2 * SCALE * h,
                   -2 * SCALE * l, -2 * SCALE * m, -2 * SCALE * h]
        m_rows += [h, h, m, h, m, l]
    ones = np.ones_like(sq)
    w_rows += [SCALE * ones] * 3 + list(_split3(SCALE * sq))
    m_rows += list(_split3(sq)) + [ones] * 3
    return (np.stack(w_rows).astype(BF16), np.stack(m_rows).astype(BF16))


def _dual_base(rows):
    """[24, N] -> [128, N] with copies at partition bases 0 and 32."""
    out = np.zeros((128, rows.shape[1]), dtype=BF16)
    out[0:K] = rows
    out[32:32 + K] = rows
    return out


def kernel(pc1, pc2):
    global _NC_CACHE, LAST_RESULTS
    p1 = np.asarray(pc1, dtype=np.float32).reshape(-1, 3)
    p2 = np.asarray(pc2, dtype=np.float32).reshape(-1, 3)
    assert p1.shape == (N1, 3) and p2.shape == (N_CORES * N_SHARD, 3)

    p1m_np = _dual_base(_prep_side(p1)[1])

    in_maps = []
    for c in range(N_CORES):
        shard = p2[c * N_SHARD:(c + 1) * N_SHARD]
        p2w_np = _dual_base(_prep_side(shard)[0])
        packed = np.concatenate([p1m_np, p2w_np], axis=1)
        in_maps.append({"inp": np.ascontiguousarray(packed)})

    if _NC_CACHE is None:
        _NC_CACHE = _build_nc()

    res = run_bass_kernel_spmd(
        _NC_CACHE, in_maps, core_ids=list(range(N_CORES)), trace=TRACE,
    )
    LAST_RESULTS = res

    # mrow per core: complete row-mins of SCALE*d2 for its 2048 pc2 pts.
    # mcol per core: partial col-mins -> min partitions, then cores.
    d2_1 = np.concatenate(
        [r["mrow"].T.reshape(-1) for r in res.results])          # [16384]
    d2_2 = np.min(
        np.stack([r["mcol"].astype(np.float32).min(axis=0)
                  for r in res.results]), axis=0)                # [16384]

    dist1 = np.sqrt(np.maximum(d2_1 / SCALE, 0.0))
    dist2 = np.sqrt(np.maximum(d2_2 / SCALE, 0.0))
    return np.asarray(dist1.mean() + dist2.mean(), dtype=np.float32)
